# revision 1
# baseline (speedup 1.0000x reference)
"""Trainium2 Bass kernel for nn_DifferentiableICP (soft-ICP, 5 iterations).

Full inputs: source [32,2048,2], target [32,2048,2], init_transformation [32,3].
Sharding: pure data parallel, 4 batch elements per core across 8 cores.

Per core / batch / iteration, the [M, N] (2048x2048) weight matrix is produced
in m-chunks of 128 rows entirely on-chip:
  cross^T[m,n] = target[m].st[n]           TensorE, K=2 matmul, f32r
  e[m,n] = exp(2*cross - |target[m]|^2)    ScalarE (the exp(-|st|^2) softmax
           factor cancels in num/den and is dropped)
  [num_x;num_y;den] += [tx;ty;1]^T @ e     TensorE, K=128, PSUM accumulate
Iteration tail (batches stacked on partitions): tc = num*recip(den),
centroid sums via ACT accum_out, H via tensor_tensor_reduce with the
centering identity, closed-form 2x2 polar factor of H^T (== V @ U^T of
jax.linalg.svd(H) for both det signs), rigid-transform compose, and st
refresh via per-partition affine ops.  No trig on device: the host provides
cos/sin of the init angle and extracts the final angle with arctan2.
"""

import contextlib

import numpy as np

import concourse.bass as bass
import concourse.mybir as mybir
import concourse.tile as tile
from concourse import bacc
from concourse import bass_utils

F32 = mybir.dt.float32
F32R = mybir.dt.float32r
AF = mybir.ActivationFunctionType
OP = mybir.AluOpType

N_CORES = 8
NB = 4          # batch elements per core
N = 2048        # source points
M = 2048        # target points
ITERS = 5


def _build_body(tc, NB, N, M, ITERS, debug_phase=0):
    nc = tc.nc
    MC = M // 128          # m-chunks
    PSF = min(1024, N)     # exp psum tile free size
    NH = N // PSF          # exp ops per m-chunk
    MMF = min(512, PSF)    # matmul moving free size
    NQ = PSF // MMF        # matmuls per psum tile
    NC4 = N // MMF         # corr matmuls per m-chunk
    ctx = contextlib.ExitStack()

    src_d = nc.dram_tensor("src_d", [NB, N, 2], F32, kind="ExternalInput").ap()
    tgt_d = nc.dram_tensor("tgt_d", [NB, M, 2], F32, kind="ExternalInput").ap()
    t0_d = nc.dram_tensor("t0_d", [NB, 8], F32, kind="ExternalInput").ap()
    out_d = nc.dram_tensor("out_d", [NB, 8], F32, kind="ExternalOutput").ap()

    pp = ctx.enter_context(tc.tile_pool(name="pers", bufs=1))
    ep = ctx.enter_context(tc.tile_pool(name="epool", bufs=3))
    xp = ctx.enter_context(tc.tile_pool(name="xpsum", bufs=2, space="PSUM"))
    np_ = ctx.enter_context(tc.tile_pool(name="ndpsum", bufs=1, space="PSUM"))

    TT = [pp.tile([2, M], F32R, tag=f"tt{b}", name=f"tt{b}") for b in range(NB)]
    TGT3 = [pp.tile([128, 3 * MC], F32R, tag=f"tg{b}", name=f"tg{b}") for b in range(NB)]
    TGTF = [pp.tile([128, 3 * MC], F32, tag=f"tgtf{b}", name=f"tgtf{b}") for b in range(NB)]
    NTSQ = [pp.tile([128, MC], F32, tag=f"nq{b}", name=f"nq{b}") for b in range(NB)]
    SQT = [pp.tile([128, 2 * MC], F32, tag=f"sqt{b}", name=f"sqt{b}") for b in range(NB)]
    SRCX = pp.tile([2 * NB, N], F32, tag="srcx", name="srcx")
    SRCY = pp.tile([2 * NB, N], F32, tag="srcy", name="srcy")
    ST8 = pp.tile([2 * NB, N], F32, tag="st8", name="st8")
    TCC = pp.tile([2 * NB, N], F32, tag="tcc", name="tcc")
    STB = [pp.tile([2, N], F32R, tag=f"stb{b}", name=f"stb{b}") for b in range(NB)]
    NDS_N = pp.tile([2 * NB, N], F32, tag="ndsn", name="ndsn")
    NDS_D = pp.tile([2 * NB, N], F32, tag="ndsd", name="ndsd")
    RC8 = pp.tile([2 * NB, N], F32, tag="rc8", name="rc8")
    STd = pp.tile([4 * NB, N], F32, tag="std", name="std")
    TCd = pp.tile([4 * NB, N], F32, tag="tcd", name="tcd")
    TS1 = pp.tile([2 * NB, N], F32, tag="ts1", name="ts1")
    T4 = pp.tile([NB, 16], F32, tag="t4", name="t4")
    SC = pp.tile([NB, 16], F32, tag="sc", name="sc")
    WS = pp.tile([NB, 32], F32, tag="ws", name="ws")
    WSB = pp.tile([NB, 16], F32, tag="wsb", name="wsb")
    RT = pp.tile([NB, 4], F32, tag="rt", name="rt")
    MKU = pp.tile([NB, 2], mybir.dt.uint8, tag="mku", name="mku")
    H16 = pp.tile([4 * NB, 1], F32, tag="h16", name="h16")
    CSTA = pp.tile([2 * NB, 1], F32, tag="csta", name="csta")
    CSTAH = pp.tile([2 * NB, 2], F32, tag="cstah", name="cstah")
    CSTB = pp.tile([2 * NB, 1], F32, tag="cstb", name="cstb")
    M1 = pp.tile([2 * NB, 1], F32, tag="m1", name="m1")
    M2 = pp.tile([2 * NB, 1], F32, tag="m2", name="m2")
    M3 = pp.tile([2 * NB, 1], F32, tag="m3", name="m3")

    nd = np_.tile([3, N], F32, tag="nd", name="nd")
    ndcp = ctx.enter_context(tc.tile_pool(name="ndcp", bufs=2))

    # ---------------- setup ----------------
    # source + transform DMAs first (st_update gates the pipeline), spread
    # across per-engine DMA queues so they run in parallel
    qs = [nc.sync, nc.sync, nc.sync, nc.sync]
    nc.sync.dma_start(T4[:, 0:8], t0_d[:, :])
    for b in range(NB):
        qs[b].dma_start(SRCX[2 * b : 2 * b + 2, :],
                        src_d[b, :, 0].unsqueeze(0).broadcast_to([2, N]))
        qs[(b + 1) % 4].dma_start(SRCY[2 * b : 2 * b + 2, :],
                                  src_d[b, :, 1].unsqueeze(0).broadcast_to([2, N]))
    for b in range(NB):
        qs[b].dma_start(TT[b][:], tgt_d[b].transpose([1, 0]).bitcast(F32R))
        dstv = TGTF[b][:].rearrange("p (c d) -> p c d", d=3)[:, :, 0:2]
        srcv = tgt_d[b].rearrange("(c p) d -> p c d", p=128)
        qs[(b + 2) % 4].dma_start(dstv, srcv)
        nc.vector.memset(TGTF[b][:, 2 : 3 * MC : 3], 1.0)
        nc.vector.tensor_copy(TGT3[b][:], TGTF[b][:])
        tv = TGTF[b][:].rearrange("p (c d) -> p c d", d=3)[:, :, 0:2]
        nc.vector.tensor_tensor(
            out=SQT[b][:].rearrange("p (c d) -> p c d", d=2), in0=tv, in1=tv, op=OP.mult
        )
        nc.vector.reduce_sum(
            out=NTSQ[b][:], in_=SQT[b][:].rearrange("p (c d) -> p c d", d=2),
            axis=mybir.AxisListType.X, negate=True,
        )

    def st_update(rb):
        nc.scalar.dma_start(M1[:], T4[:, rb + 0 : rb + 2])
        nc.scalar.dma_start(M2[:], T4[:, rb + 2 : rb + 6 : 3])
        nc.scalar.dma_start(M3[:], T4[:, rb + 3 : rb + 5])
        NH2 = N // 2
        for h in range(2):
            sl = slice(h * NH2, (h + 1) * NH2)
            nc.scalar.activation(TS1[:, sl], SRCX[:, sl], AF.Identity,
                                 bias=M3[:, 0:1], scale=M1[:, 0:1])
            nc.vector.scalar_tensor_tensor(out=ST8[:, sl], in0=SRCY[:, sl],
                                           scalar=M2[:, 0:1], in1=TS1[:, sl],
                                           op0=OP.mult, op1=OP.add,
                                           accum_out=CSTAH[:, h : h + 1])
        nc.vector.tensor_add(CSTA[:], CSTAH[:, 0:1], CSTAH[:, 1:2])
        for b in range(NB):
            qs[b].dma_start(STB[b][:], ST8[2 * b : 2 * b + 2, :].bitcast(F32R))

    st_update(0)

    invN = 1.0 / float(N)

    for it in range(ITERS):
        rb = 8 * (it % 2)
        wb = 8 * ((it + 1) % 2)
        # ---------------- main phase ----------------
        for b in range(NB):
            for mc in range(MC):
                e_t = ep.tile([128, N], F32R, tag="e", name="e_t")
                for h in range(NH):
                    xps = xp.tile([128, PSF], F32, tag="x", name="xps")
                    for q in range(NQ):
                        nc.tensor.matmul(
                            xps[:, MMF * q : MMF * q + MMF],
                            lhsT=TT[b][:, 128 * mc : 128 * mc + 128],
                            rhs=STB[b][:, PSF * h + MMF * q : PSF * h + MMF * q + MMF],
                            start=True, stop=True,
                        )
                    nc.scalar.activation(
                        e_t[:, PSF * h : PSF * h + PSF], xps[:],
                        AF.Exp, bias=NTSQ[b][:, mc : mc + 1], scale=2.0,
                    )
                for c2 in range(NC4):
                    nc.tensor.matmul(
                        nd[:, MMF * c2 : MMF * c2 + MMF],
                        lhsT=TGT3[b][:, 3 * mc : 3 * mc + 3],
                        rhs=e_t[:, MMF * c2 : MMF * c2 + MMF],
                        start=(mc == 0), stop=(mc == MC - 1),
                    )
            ndc = ndcp.tile([3, N], F32, tag="ndc", name="ndc")
            NDB = min(512, N)
            for bk in range(N // NDB):
                nc.vector.tensor_copy(ndc[:, NDB * bk : NDB * bk + NDB],
                                      nd[:, NDB * bk : NDB * bk + NDB])
            nc.sync.dma_start(NDS_N[2 * b : 2 * b + 2, :], ndc[0:2, :])
            nc.sync.dma_start(NDS_D[2 * b : 2 * b + 1, :], ndc[2:3, :])
            nc.sync.dma_start(NDS_D[2 * b + 1 : 2 * b + 2, :], ndc[2:3, :])
        # ---------------- tail ----------------
        if debug_phase == 99:
            continue
        if debug_phase == 1:
            nc.sync.dma_start(out_d[:, 0:8], NDS_N[0:NB, 0:8])
            ctx.close()
            return
        nc.vector.reciprocal_approx_accurate(out=RC8[:], in_=NDS_D[:],
                                             scratch=TS1[0 : 2 * NB, :])
        nc.vector.tensor_tensor(out=TCC[:], in0=NDS_N[:], in1=RC8[:], op=OP.mult)
        if debug_phase == 21:
            nc.sync.dma_start(out_d[:, 0:8], TCC[0:NB, 0:8])
            ctx.close()
            return
        scr = ep.tile([4 * NB, N], F32, tag="e", name="escr")
        nc.scalar.activation(scr[0 : 2 * NB, :], ST8[:], AF.Copy, accum_out=CSTA[:])
        nc.scalar.activation(scr[0 : 2 * NB, :], TCC[:], AF.Copy, accum_out=CSTB[:])
        if debug_phase == 22:
            nc.sync.dma_start(out_d[:, 0:2], CSTA[0 : 2 * NB, :])
            nc.sync.dma_start(out_d[:, 2:4], CSTB[0 : 2 * NB, :])
            ctx.close()
            return
        for b in range(NB):
            nc.sync.dma_start(
                STd[4 * b : 4 * b + 4, :],
                ST8[2 * b : 2 * b + 2, :].unsqueeze(1).broadcast_to([2, 2, N]),
            )
            for r in range(2):
                nc.scalar.dma_start(TCd[4 * b + 2 * r : 4 * b + 2 * r + 2, :],
                                    TCC[2 * b : 2 * b + 2, :])
        if debug_phase == 23:
            nc.sync.dma_start(out_d[:, 0:4], STd[0:NB, 0:4])
            nc.sync.dma_start(out_d[:, 4:8], TCd[0:NB, 0:4])
            ctx.close()
            return
        scr2 = ep.tile([4 * NB, N], F32, tag="e", name="escr2")
        nc.vector.scalar_tensor_tensor(
            out=scr2[:], in0=STd[:], scalar=0.0, in1=TCd[:],
            op0=OP.bypass, op1=OP.mult, accum_out=H16[:],
        )
        if debug_phase == 2:
            nc.sync.dma_start(out_d[:, 0:4], H16[0 : 4 * NB, :])
            ctx.close()
            return
        # gather per-batch scalars: SC = (Hxx,Hxy,Hyx,Hyy, Ssx,Ssy, Stx,Sty)
        nc.scalar.dma_start(SC[:, 0:4], H16[:])
        nc.scalar.dma_start(SC[:, 4:6], CSTA[:])
        nc.scalar.dma_start(SC[:, 6:8], CSTB[:])
        v = nc.vector
        # centering correction: H -= outer(Ss, St)/N  -> WS[12:16]
        in0 = SC[:, 4:6].unsqueeze(2).broadcast_to([NB, 2, 2])
        in1 = SC[:, 6:8].unsqueeze(1).broadcast_to([NB, 2, 2])
        v.tensor_tensor(out=WS[:, 0:4].rearrange("p (a c) -> p a c", c=2), in0=in0, in1=in1, op=OP.mult)
        v.tensor_scalar(out=WS[:, 8:12], in0=WS[:, 0:4], scalar1=invN, scalar2=None, op0=OP.mult)
        v.tensor_tensor(out=WS[:, 12:16], in0=SC[:, 0:4], in1=WS[:, 8:12], op=OP.subtract)
        # A = H^T; (p, q, r, s2) at WS[16:20]
        v.tensor_tensor(out=WS[:, 16:17], in0=WS[:, 12:13], in1=WS[:, 15:16], op=OP.add)
        v.tensor_tensor(out=WS[:, 17:18], in0=WS[:, 14:15], in1=WS[:, 13:14], op=OP.subtract)
        v.tensor_tensor(out=WS[:, 18:19], in0=WS[:, 12:13], in1=WS[:, 15:16], op=OP.subtract)
        v.tensor_tensor(out=WS[:, 19:20], in0=WS[:, 14:15], in1=WS[:, 13:14], op=OP.add)
        # det at WS[22]
        v.tensor_tensor(out=WS[:, 20:21], in0=WS[:, 12:13], in1=WS[:, 15:16], op=OP.mult)
        v.tensor_tensor(out=WS[:, 21:22], in0=WS[:, 13:14], in1=WS[:, 14:15], op=OP.mult)
        v.tensor_tensor(out=WS[:, 22:23], in0=WS[:, 20:21], in1=WS[:, 21:22], op=OP.subtract)
        # hypots^2 at WS[28:30]; rsqrt via exp(-0.5*ln)
        v.tensor_tensor(out=WS[:, 24:28], in0=WS[:, 16:20], in1=WS[:, 16:20], op=OP.mult)
        v.tensor_tensor(out=WS[:, 28:30], in0=WS[:, 24:27:2], in1=WS[:, 25:28:2], op=OP.add)
        nc.scalar.activation(WS[:, 30:32], WS[:, 28:30], AF.Ln)
        nc.scalar.activation(WSB[:, 0:2], WS[:, 30:32], AF.Exp, scale=-0.5)
        v.tensor_tensor(
            out=WSB[:, 2:6].rearrange("p (a c) -> p a c", c=2),
            in0=WS[:, 16:20].rearrange("p (a c) -> p a c", c=2),
            in1=WSB[:, 0:2].unsqueeze(2).broadcast_to([NB, 2, 2]),
            op=OP.mult,
        )
        v.tensor_scalar(out=WSB[:, 6:8], in0=WS[:, 22:23].broadcast_to([NB, 2]),
                        scalar1=0.0, scalar2=None, op0=OP.is_gt)
        v.tensor_scalar(out=MKU[:], in0=WS[:, 22:23].broadcast_to([NB, 2]),
                        scalar1=0.0, scalar2=None, op0=OP.is_gt)
        v.tensor_copy(WSB[:, 8:10], WSB[:, 4:6])
        v.copy_predicated(WSB[:, 8:10], MKU[:], WSB[:, 2:4])
        v.tensor_scalar(out=WSB[:, 10:11], in0=WSB[:, 6:7], scalar1=2.0, scalar2=-1.0,
                        op0=OP.mult, op1=OP.add)
        v.tensor_scalar(out=WSB[:, 11:12], in0=WSB[:, 10:11], scalar1=-1.0, scalar2=None, op0=OP.mult)
        v.tensor_copy(RT[:, 0:2], WSB[:, 8:10])
        v.tensor_tensor(out=RT[:, 2:3], in0=WSB[:, 9:10], in1=WSB[:, 11:12], op=OP.mult)
        v.tensor_tensor(out=RT[:, 3:4], in0=WSB[:, 8:9], in1=WSB[:, 10:11], op=OP.mult)
        # t_delta = (St - R @ Ss)/N -> WS[4:6]
        v.tensor_tensor(
            out=WSB[:, 12:16].rearrange("p (a c) -> p a c", c=2),
            in0=RT[:].rearrange("p (a c) -> p a c", c=2),
            in1=SC[:, 4:6].unsqueeze(1).broadcast_to([NB, 2, 2]),
            op=OP.mult,
        )
        v.tensor_tensor(out=WS[:, 0:2], in0=WSB[:, 12:15:2], in1=WSB[:, 13:16:2], op=OP.add)
        v.tensor_tensor(out=WS[:, 2:4], in0=SC[:, 6:8], in1=WS[:, 0:2], op=OP.subtract)
        v.tensor_scalar(out=WS[:, 4:6], in0=WS[:, 2:4], scalar1=invN, scalar2=None, op0=OP.mult)
        # (c^, s^) = first column of R (unit norm up to rounding; the
        # reference renormalizes via cos/sin(atan2) which differs by ~1e-7)
        # compose T_new = delta @ T_old
        v.tensor_copy(WSB[:, 14:15], RT[:, 2:3])
        v.tensor_copy(WSB[:, 15:16], RT[:, 0:1])
        v.tensor_tensor(out=WS[:, 12:14], in0=RT[:, 0:4:2], in1=T4[:, rb : rb + 2], op=OP.mult)
        v.tensor_tensor(out=WS[:, 14:16], in0=WSB[:, 14:16], in1=T4[:, rb : rb + 2], op=OP.mult)
        v.tensor_tensor(out=T4[:, wb : wb + 1], in0=WS[:, 12:13], in1=WS[:, 13:14], op=OP.subtract)
        v.tensor_copy(T4[:, wb + 5 : wb + 6], T4[:, wb : wb + 1])
        v.tensor_tensor(out=T4[:, wb + 1 : wb + 2], in0=WS[:, 14:15], in1=WS[:, 15:16], op=OP.add)
        v.tensor_scalar(out=T4[:, wb + 2 : wb + 3], in0=T4[:, wb + 1 : wb + 2],
                        scalar1=-1.0, scalar2=None, op0=OP.mult)
        v.tensor_tensor(out=WS[:, 16:18], in0=RT[:, 0:4:2], in1=T4[:, rb + 3 : rb + 5], op=OP.mult)
        v.tensor_tensor(out=WS[:, 18:20], in0=WSB[:, 14:16], in1=T4[:, rb + 3 : rb + 5], op=OP.mult)
        v.tensor_tensor(out=WS[:, 20:21], in0=WS[:, 16:17], in1=WS[:, 17:18], op=OP.subtract)
        v.tensor_tensor(out=T4[:, wb + 3 : wb + 4], in0=WS[:, 20:21], in1=WS[:, 4:5], op=OP.add)
        v.tensor_tensor(out=WS[:, 21:22], in0=WS[:, 18:19], in1=WS[:, 19:20], op=OP.add)
        v.tensor_tensor(out=T4[:, wb + 4 : wb + 5], in0=WS[:, 21:22], in1=WS[:, 5:6], op=OP.add)

        if it < ITERS - 1:
            st_update(wb)

    fb = 8 * (ITERS % 2)
    if debug_phase == 99:
        nc.sync.dma_start(out_d[:, 0:8], NDS_N[0:NB, 0:8])
    else:
        nc.sync.dma_start(out_d[:, 0:5], T4[:, fb : fb + 5])
    ctx.close()


_CACHED_NC = None


def _pin_exp_to_combined_set():
    """Force the act-table chooser to place Exp in natural_log_exp_and_others
    (which also holds Ln/Identity) so the unrolled loop needs one table load
    total.  Keeps the dict ORDER intact: act_func_set_id is the positional
    index into act_info.json, so only func membership may be edited."""
    import concourse.bacc as _bm
    from concourse.hw_specs import get_activation_tables as _gat
    if getattr(_bm, "_icp_exp_pin", False):
        return
    def _gat2(arch):
        t = _gat(arch)
        if "natural_log_exp_and_others" not in t:
            return t
        exp = mybir.ActivationFunctionType.Exp
        out = {}
        for name, funcs in t.items():
            if name != "natural_log_exp_and_others" and exp in funcs:
                funcs = set(funcs) - {exp}
            out[name] = funcs
        return out
    _bm.get_activation_tables = _gat2
    _bm._icp_exp_pin = True


def _get_nc():
    global _CACHED_NC
    if _CACHED_NC is None:
        _pin_exp_to_combined_set()
        nc = bacc.Bacc("TRN2", target_bir_lowering=False, debug=False,
                       num_devices=N_CORES)
        with tile.TileContext(nc) as tc:
            _build_body(tc, NB, N, M, ITERS)
        nc.compile()
        _CACHED_NC = nc
    return _CACHED_NC


def _host_pre(init_trans):
    th = init_trans[:, 0].astype(np.float64)
    c, s = np.cos(th), np.sin(th)
    z = np.zeros_like(c)
    return np.stack([c, s, -s, init_trans[:, 1], init_trans[:, 2], c, z, z],
                    axis=1).astype(np.float32)


def run_kernel(source, target, init_transformation, trace=False):
    source = np.ascontiguousarray(np.asarray(source, dtype=np.float32))
    target = np.ascontiguousarray(np.asarray(target, dtype=np.float32))
    init = np.ascontiguousarray(np.asarray(init_transformation, dtype=np.float32))
    B = source.shape[0]
    assert B == N_CORES * NB, (B, N_CORES, NB)
    nc = _get_nc()
    in_maps = []
    for c in range(N_CORES):
        sl = slice(c * NB, (c + 1) * NB)
        in_maps.append({
            "src_d": np.ascontiguousarray(source[sl]),
            "tgt_d": np.ascontiguousarray(target[sl]),
            "t0_d": _host_pre(init[sl]),
        })
    res = bass_utils.run_bass_kernel_spmd(
        nc, in_maps, core_ids=list(range(N_CORES)), trace=trace,
    )
    outs = []
    for c in range(N_CORES):
        t4 = np.asarray(res.results[c]["out_d"])
        th = np.arctan2(t4[:, 1], t4[:, 0])
        outs.append(np.stack([th, t4[:, 3], t4[:, 4]], axis=1))
    full = np.concatenate(outs, axis=0).astype(np.float32)
    return full, res.exec_time_ns


def kernel(source, target, init_transformation):
    out, _ = run_kernel(source, target, init_transformation)
    return out


def measure_exec_ns(source, target, init_transformation, reps=30):
    """Amortized per-execution time via async dispatch of the sharded jit."""
    import time
    import jax
    from jax.sharding import Mesh, PartitionSpec
    from jax.experimental.shard_map import shard_map
    from concourse import bass2jax, mybir as _mybir

    nc = _get_nc()
    bass2jax.install_neuronx_cc_hook()
    source = np.asarray(source, dtype=np.float32)
    target = np.asarray(target, dtype=np.float32)
    init = np.asarray(init_transformation, dtype=np.float32)

    in_names, out_names, out_avals, zero_outs = [], [], [], []
    partition_name = nc.partition_id_tensor.name if nc.partition_id_tensor else None
    for alloc in nc.m.functions[0].allocations:
        if not isinstance(alloc, _mybir.MemoryLocationSet):
            continue
        name = alloc.memorylocations[0].name
        if alloc.kind == "ExternalInput":
            if name != partition_name:
                in_names.append(name)
        elif alloc.kind == "ExternalOutput":
            out_names.append(name)
            shape = tuple(alloc.tensor_shape)
            dtype = _mybir.dt.np(alloc.dtype)
            out_avals.append(jax.core.ShapedArray(shape, dtype))
            zero_outs.append(np.zeros(shape, dtype))
    n_params = len(in_names)
    all_in_names = in_names + out_names
    if partition_name is not None:
        all_in_names = all_in_names + [partition_name]

    def _body(*args):
        operands = list(args)
        if partition_name is not None:
            operands.append(bass2jax.partition_id_tensor())
        outs = bass2jax._bass_exec_p.bind(
            *operands,
            out_avals=tuple(out_avals),
            in_names=tuple(all_in_names),
            out_names=tuple(out_names),
            lowering_input_output_aliases=(),
            sim_require_finite=True,
            sim_require_nnan=True,
            nc=nc,
        )
        return tuple(outs)

    devices = jax.devices()[:N_CORES]
    mesh = Mesh(np.asarray(devices), ("core",))
    n_outs = len(out_names)
    in_specs = (PartitionSpec("core"),) * (n_params + n_outs)
    out_specs = (PartitionSpec("core"),) * n_outs
    sharded = jax.jit(
        shard_map(_body, mesh=mesh, in_specs=in_specs, out_specs=out_specs,
                  check_rep=False),
        keep_unused=True,
    )
    in_map_by_name = {}
    for c in range(N_CORES):
        sl = slice(c * NB, (c + 1) * NB)
        m = {"src_d": source[sl], "tgt_d": target[sl], "t0_d": _host_pre(init[sl])}
        for k, v in m.items():
            in_map_by_name.setdefault(k, []).append(np.ascontiguousarray(v))
    concat_in = [np.concatenate(in_map_by_name[nm], axis=0) for nm in in_names]
    concat_zeros = [np.zeros((N_CORES * z.shape[0], *z.shape[1:]), z.dtype)
                    for z in zero_outs]
    args = concat_in + concat_zeros
    out = sharded(*args)
    jax.block_until_ready(out)
    t0 = time.perf_counter()
    outs = [sharded(*args) for _ in range(reps)]
    jax.block_until_ready(outs)
    t1 = time.perf_counter()
    return int((t1 - t0) / reps * 1e9)


def measure_kernel_ns(source, target, init_transformation, reps=40):
    """Device-time estimate via ITERS=10 vs ITERS=5 delta (cancels the axon
    dispatch overhead, which dominates amortized per-call wall time here)."""
    import time
    import jax
    from jax.sharding import Mesh, PartitionSpec
    from jax.experimental.shard_map import shard_map
    from concourse import bass2jax, mybir as _mybir

    def build(iters):
        nc = bacc.Bacc("TRN2", target_bir_lowering=False, debug=False,
                       num_devices=N_CORES)
        with tile.TileContext(nc) as tc:
            _build_body(tc, NB, N, M, iters)
        nc.compile()
        return nc

    argmap = {
        "src_d": np.ascontiguousarray(np.asarray(source, np.float32)),
        "tgt_d": np.ascontiguousarray(np.asarray(target, np.float32)),
        "t0_d": np.concatenate([
            _host_pre(np.asarray(init_transformation, np.float32)[c * NB:(c + 1) * NB])
            for c in range(N_CORES)]),
    }

    def measure(nc):
        bass2jax.install_neuronx_cc_hook()
        in_names, out_names, out_avals, zero_outs = [], [], [], []
        pn = nc.partition_id_tensor.name if nc.partition_id_tensor else None
        for alloc in nc.m.functions[0].allocations:
            if not isinstance(alloc, _mybir.MemoryLocationSet):
                continue
            name = alloc.memorylocations[0].name
            if alloc.kind == "ExternalInput":
                if name != pn:
                    in_names.append(name)
            elif alloc.kind == "ExternalOutput":
                out_names.append(name)
                out_avals.append(jax.core.ShapedArray(
                    tuple(alloc.tensor_shape), _mybir.dt.np(alloc.dtype)))
                zero_outs.append(np.zeros(tuple(alloc.tensor_shape),
                                          _mybir.dt.np(alloc.dtype)))
        all_in = in_names + out_names + ([pn] if pn else [])

        def _body(*args):
            ops = list(args)
            if pn:
                ops.append(bass2jax.partition_id_tensor())
            return tuple(bass2jax._bass_exec_p.bind(
                *ops, out_avals=tuple(out_avals), in_names=tuple(all_in),
                out_names=tuple(out_names), lowering_input_output_aliases=(),
                sim_require_finite=True, sim_require_nnan=True, nc=nc))

        devices = jax.devices()[:N_CORES]
        mesh = Mesh(np.asarray(devices), ("core",))
        nin, nout = len(in_names), len(out_names)
        sharded = jax.jit(shard_map(
            _body, mesh=mesh, in_specs=(PartitionSpec("core"),) * (nin + nout),
            out_specs=(PartitionSpec("core"),) * nout, check_rep=False),
            keep_unused=True)
        args = [argmap[n] for n in in_names]
        args += [np.zeros((N_CORES * z.shape[0], *z.shape[1:]), z.dtype)
                 for z in zero_outs]
        o = sharded(*args)
        jax.block_until_ready(o)

        def timed():
            t0 = time.perf_counter()
            outs = [sharded(*args) for _ in range(reps)]
            jax.block_until_ready(outs)
            return (time.perf_counter() - t0) / reps
        return timed

    m5 = measure(_get_nc())
    m10 = measure(build(2 * ITERS))
    t5s, t10s = [], []
    m5(); m10()  # warm both
    for _ in range(6):
        t5s.append(m5())
        t10s.append(m10())
    t5 = float(np.median(t5s))
    t10 = float(np.median(t10s))
    kernel_ns = max(0.0, (t10 - t5) * 1e9)
    return int(kernel_ns), int(t5 * 1e9)

